# revision 9
# baseline (speedup 1.0000x reference)
"""GRU kernel for Trainium2, 8-core data-parallel over batch.

Contract: kernel(**inputs) takes the FULL inputs from setup_inputs()
(xs [64,1024,256], Wz/Wr/Wh [768,512], bz/br/bh [512]) and returns the
full output (hs [64,1024,512], h_last [64,512]).

Per-core layout (batch shard of 8 rows):
  - Input projections xs @ Wx (+bias) are computed ON CHIP, fused with the
    recurrence: results live in a transposed SBUF ring (xT chunks,
    [128 n-part, 12 n-chunks, 128 (t,b)-cols] bf16), never touching DRAM.
  - Recurrence state is kept twice per step:
      hT   [128, 32] f32  : hT[p, 8k+b] = h[b, 128k+p]   (stationary form)
      hgrp [128,128] f32  : hgrp[32g+b, c] = h[b, 128g+c] (elementwise form)
  - Gate preactivations: 4 PE column-groups, one 128-wide slice each.
    x_t is injected into PSUM with an extra matmul (stationary = xT slice,
    moving = 128x128 identity), so no DVE add sits on the serial chain.
  - Blend increment e = z*(hc-h) is PE-transposed once per step to update hT.
"""

import sys

try:
    import concourse.bass as bass  # noqa: F401
except ImportError:  # pragma: no cover - fallback for bare environments
    for p in ("/opt/trn_rl_repo", "/root/.axon_site/_ro/trn_rl_repo"):
        if p not in sys.path:
            sys.path.append(p)

import numpy as np

import concourse.bass as bass
import concourse.mybir as mybir
import concourse.tile as tile
from concourse import bacc
from concourse.bass_utils import run_bass_kernel_spmd
from concourse.masks import make_identity

F32 = mybir.dt.float32
F32R = mybir.dt.float32r
BF16 = mybir.dt.bfloat16

N_CORES = 8
B = 8          # batch rows per core
I = 256        # input dim
H = 512        # hidden dim
G3 = 3 * H     # concatenated gate dim (z|r|h)
NG = G3 // 128     # 12 n-chunks of 128
KC = H // 128      # 4 k-chunks of the hidden contraction
IC = I // 128      # 2 k-chunks of the input contraction
CH = 16            # timesteps per phase-1 chunk (16*8 = 128 m-cols)
LOOKAHEAD = 7      # phase-1 chunks emitted ahead of consumption
RING_BUFS = LOOKAHEAD + 1

_cache = {}


def build_program(T):
    NCHUNK = T // CH
    nc = bacc.Bacc("TRN2", target_bir_lowering=False, debug=False,
                   num_devices=N_CORES)

    xs = nc.dram_tensor("xs", [T, B, I], F32, kind="ExternalInput")
    wx = nc.dram_tensor("wx", [I, G3], F32, kind="ExternalInput")
    wh = nc.dram_tensor("wh", [H, G3], F32, kind="ExternalInput")
    bias = nc.dram_tensor("bias", [G3], F32, kind="ExternalInput")
    hs = nc.dram_tensor("hs", [B, T, H], F32, kind="ExternalOutput")
    hlast = nc.dram_tensor("hlast", [B, H], F32, kind="ExternalOutput")

    with tile.TileContext(nc) as tc:
        with (
            tc.tile_pool(name="const", bufs=1) as const,
            tc.tile_pool(name="wtmp", bufs=2) as wtmp_pool,
            tc.tile_pool(name="xring", bufs=RING_BUFS) as xring,
            tc.tile_pool(name="step", bufs=2) as step,
            tc.tile_pool(name="ps_state", bufs=1, space="PSUM") as ps_state,
            tc.tile_pool(name="ps_t", bufs=3, space="PSUM") as ps_t,
            tc.tile_pool(name="ps_p1", bufs=2, space="PSUM") as ps_p1,
        ):
            # ---- constants / weights -------------------------------------
            ident_f32 = const.tile([128, 128], F32, name="ident_f32")
            make_identity(nc, ident_f32[:])
            ident_bf16 = const.tile([128, 128], BF16, name="ident_bf16")
            make_identity(nc, ident_bf16[:])

            bias_t = const.tile([128, NG], F32, name="bias_t")
            nc.sync.dma_start(bias_t[:], bias.ap().rearrange("(j p) -> p j", p=128))

            wh_sb = const.tile([128, KC, G3], BF16, name="wh_sb")
            for k in range(KC):
                wtmp = wtmp_pool.tile([128, G3], F32, name="wtmp", tag="wtmp")
                nc.sync.dma_start(wtmp[:], wh.ap()[k * 128:(k + 1) * 128, :])
                nc.vector.tensor_copy(out=wh_sb[:, k, :], in_=wtmp[:])
            wx_sb = const.tile([128, IC, G3], BF16, name="wx_sb")
            for k in range(IC):
                wtmp = wtmp_pool.tile([128, G3], F32, name="wtmp", tag="wtmp")
                nc.sync.dma_start(wtmp[:], wx.ap()[k * 128:(k + 1) * 128, :])
                nc.vector.tensor_copy(out=wx_sb[:, k, :], in_=wtmp[:])

            # ---- persistent state ----------------------------------------
            hT = [const.tile([128, KC, B], F32, name=f"hT{p}") for p in range(2)]
            hTb = [const.tile([128, KC, B], BF16, name=f"hTb{p}") for p in range(2)]
            hgrp = [const.tile([128, 128], F32, name=f"hgrp{p}") for p in range(2)]
            nc.vector.memset(hT[0][:], 0.0)
            nc.vector.memset(hTb[0][:], 0.0)
            nc.vector.memset(hgrp[0][:], 0.0)

            r_ps = ps_state.tile([128, 128], F32, name="r_ps")
            z_ps = ps_state.tile([128, 128], F32, name="z_ps")
            hc_ps = ps_state.tile([128, 128], F32, name="hc_ps")
            nc.vector.memset(r_ps[:], 0.0)
            nc.vector.memset(z_ps[:], 0.0)
            nc.vector.memset(hc_ps[:], 0.0)

            xchunks = [None] * NCHUNK

            # ---- phase 1: fused input projection -------------------------
            def emit_p1_dma(c):
                xs_sb = step.tile([128, I], F32, name="xs_sb", tag="xs_sb")
                nc.sync.dma_start(
                    xs_sb[:],
                    xs.ap()[c * CH:(c + 1) * CH].rearrange("t b i -> (t b) i"),
                )
                xsT_sb = step.tile([128, I], BF16, name="xsT_sb", tag="xsT_sb")
                xchunks[c] = (xs_sb, xsT_sb,
                              xring.tile([128, NG, 128], BF16, name="xchunk", tag="xchunk"))

            def emit_p1_transpose(c, k):
                xs_sb, xsT_sb, _ = xchunks[c]
                tp = ps_t.tile([128, 128], F32, name="tpool", tag="tpool")
                nc.tensor.transpose(
                    tp[:], xs_sb[:, k * 128:(k + 1) * 128], ident_f32[:])
                nc.vector.tensor_copy(out=xsT_sb[:, k * 128:(k + 1) * 128], in_=tp[:])

            def emit_p1_gemm(c, j):
                _, xsT_sb, xt = xchunks[c]
                p1 = ps_p1.tile([128, 128], F32, name="p1", tag="p1")
                for k in range(IC):
                    nc.tensor.matmul(
                        p1[:],
                        wx_sb[:, k, j * 128:(j + 1) * 128],
                        xsT_sb[:, k * 128:(k + 1) * 128],
                        start=(k == 0), stop=(k == IC - 1),
                    )
                nc.vector.tensor_scalar(
                    xt[:, j, :], p1[:], bias_t[:, j:j + 1], None,
                    mybir.AluOpType.add,
                )

            def emit_p1_piece(c, s):
                # schedule: s=0 dma, s=1/2 transpose, s=3..14 gemm chunks
                if c >= NCHUNK:
                    return
                if s == 0:
                    emit_p1_dma(c)
                elif s in (1, 2):
                    emit_p1_transpose(c, s - 1)
                elif 3 <= s <= 14:
                    emit_p1_gemm(c, s - 3)

            def emit_p1_chunk(c):
                for s in range(CH):
                    emit_p1_piece(c, s)

            for c in range(min(LOOKAHEAD, NCHUNK)):
                emit_p1_chunk(c)

            # ---- recurrence ----------------------------------------------
            def gate_mms(ps, xt, mcol, jbase, hstat_tile):
                # one gate: 4 column-groups, each 1 inject + KC weight MMs
                for g in range(4):
                    j = jbase + g
                    out = ps[32 * g:32 * g + B, :]
                    nc.tensor.matmul(
                        out, xt[:, j, mcol:mcol + B], ident_bf16[:],
                        start=True, stop=False, tile_position=(0, 32 * g),
                    )
                    for k in range(KC):
                        nc.tensor.matmul(
                            out,
                            hstat_tile[:, k, :],
                            wh_sb[:, k, j * 128:(j + 1) * 128],
                            start=False, stop=(k == KC - 1),
                            tile_position=(0, 32 * g),
                        )

            for t in range(T):
                p = t % 2
                c, dt = divmod(t, CH)
                mcol = B * dt
                _, _, xt = xchunks[c]
                hT_cur, hT_new = hT[p], hT[1 - p]
                hg_cur, hg_new = hgrp[p], hgrp[1 - p]

                # r and z preactivations
                gate_mms(r_ps, xt, mcol, 4, hTb[p])
                gate_mms(z_ps, xt, mcol, 0, hTb[p])

                r_s = step.tile([128, 128], F32, name="r_s", tag="r_s")
                nc.scalar.activation(r_s[:], r_ps[:],
                                     mybir.ActivationFunctionType.Sigmoid)
                z_s = step.tile([128, 128], F32, name="z_s", tag="z_s")
                nc.scalar.activation(z_s[:], z_ps[:],
                                     mybir.ActivationFunctionType.Sigmoid)

                rh = step.tile([128, 128], BF16, name="rh", tag="rh")
                nc.vector.tensor_mul(out=rh[:], in0=r_s[:], in1=hg_cur[:])

                rhT_ps = ps_t.tile([128, 128], BF16, name="tpool", tag="tpool")
                nc.tensor.transpose(rhT_ps[:], rh[:], ident_bf16[:])
                rhT = step.tile([128, KC, B], BF16, name="rhT", tag="rhT")
                nc.vector.tensor_copy(
                    out=rhT[:],
                    in_=rhT_ps[:].rearrange("p (g z b) -> p g z b", g=4, b=B)[:, :, 0, :],
                )

                # candidate preactivation
                gate_mms(hc_ps, xt, mcol, 8, rhT)
                hc_s = step.tile([128, 128], F32, name="hc_s", tag="hc_s")
                nc.scalar.activation(hc_s[:], hc_ps[:],
                                     mybir.ActivationFunctionType.Tanh)

                d = step.tile([128, 128], F32, name="d", tag="d")
                nc.vector.tensor_sub(out=d[:], in0=hc_s[:], in1=hg_cur[:])
                e = step.tile([128, 128], F32, name="e", tag="e")
                nc.vector.tensor_mul(out=e[:], in0=z_s[:], in1=d[:])

                eT_ps = ps_t.tile([128, 128], F32, name="tpool", tag="tpool")
                nc.tensor.transpose(eT_ps[:], e[:], ident_f32[:])
                nc.vector.tensor_tensor(
                    hT_new[:],
                    hT_cur[:],
                    eT_ps[:].rearrange("p (g z b) -> p g z b", g=4, b=B)[:, :, 0, :],
                    mybir.AluOpType.add,
                )
                nc.vector.tensor_copy(out=hTb[1 - p][:], in_=hT_new[:])
                nc.gpsimd.tensor_add(out=hg_new[:], in0=hg_cur[:], in1=e[:])

                for g in range(4):
                    nc.sync.dma_start(
                        hs.ap()[:, t, 128 * g:128 * (g + 1)],
                        hg_new[32 * g:32 * g + B, :],
                    )

                emit_p1_piece(c + LOOKAHEAD, dt)

            for g in range(4):
                nc.sync.dma_start(
                    hlast.ap()[:, 128 * g:128 * (g + 1)],
                    hgrp[T % 2][32 * g:32 * g + B, :],
                )

    nc.compile()
    return nc


def _get_program(T):
    if T not in _cache:
        _cache[T] = build_program(T)
    return _cache[T]


def kernel(xs, Wz, bz, Wr, br, Wh, bh, trace=False, tmpdir=None):
    xs = np.asarray(xs, dtype=np.float32)
    Bfull, T, _ = xs.shape
    assert Bfull == N_CORES * B

    wx_cat = np.ascontiguousarray(
        np.concatenate([Wz[:I], Wr[:I], Wh[:I]], axis=1), dtype=np.float32)
    wh_cat = np.ascontiguousarray(
        np.concatenate([Wz[I:], Wr[I:], Wh[I:]], axis=1), dtype=np.float32)
    bias_cat = np.ascontiguousarray(
        np.concatenate([bz, br, bh]), dtype=np.float32)

    nc = _get_program(T)
    in_maps = [
        {
            "xs": np.ascontiguousarray(xs[c * B:(c + 1) * B].transpose(1, 0, 2)),
            "wx": wx_cat,
            "wh": wh_cat,
            "bias": bias_cat,
        }
        for c in range(N_CORES)
    ]
    res = run_bass_kernel_spmd(nc, in_maps, list(range(N_CORES)),
                               trace=trace, tmpdir=tmpdir)
    hs = np.concatenate([res.results[c]["hs"] for c in range(N_CORES)], axis=0)
    hl = np.concatenate([res.results[c]["hlast"] for c in range(N_CORES)], axis=0)
    kernel.last_exec_time_ns = res.exec_time_ns
    return hs, hl


kernel.last_exec_time_ns = None
